# revision 2
# baseline (speedup 1.0000x reference)
"""BurstGNN Trainium2 kernel — single-launch, 8-core SPMD.

Sharding: nodes/edges partitioned by dst across 8 cores; small weights
replicated. All heavy data stays on device:

- encoder (num/cat MLP -> x) computed on device from per-core shards
- FAConv x2: per-edge h[src] gathered from a device-resident replicated
  table via indirect DMA; alpha = tanh(al_src + ar_dst) * norm computed
  on device; scatter-sum via one-hot matmuls accumulating in PSUM
- halo exchange: on-device AllGather of the per-core [al | h] bf16 table
  between layers (3 collectives total)
- ragged per-user segment sums (re_index resolved host-side into static
  slot metadata) + final MLP on device

Host does only: index preprocessing (slot packing), input shard layout,
and the trivial final unshard. One launch; ~45MB host->device traffic.
"""

import sys

sys.path.insert(0, "/opt/trn_rl_repo")

import ml_dtypes
import numpy as np

import concourse.bass as bass
import concourse.bacc as bacc
import concourse.mybir as mybir
import concourse.tile as tile

F32 = mybir.dt.float32
BF16 = mybir.dt.bfloat16
I32 = mybir.dt.int32
I8 = mybir.dt.int8
AF = mybir.ActivationFunctionType
OP = mybir.AluOpType
AX = mybir.AxisListType
ds = bass.ds

EPS = 0.1
SLOPE = 0.01


class Cfg:
    def __init__(self, N=200000, E=1600000, U=20000, NUMP=20, CATP=12):
        self.N, self.E, self.U = N, E, U
        self.NUMP, self.CATP = NUMP, CATP
        self.C = 8
        self.D = 64
        self.NS = N // self.C                      # nodes per core
        self.T = (self.NS + 127) // 128            # dst tiles per core
        self.NSP = self.T * 128                    # padded rows per core
        self.TBLR = self.C * self.NSP              # global padded rows
        self.UPC = U // self.C                     # users per core
        self.UW = (self.UPC + 127) // 128          # user tiles per core
        self.UPCP = self.UW * 128


def _ap(base, dims, extra_off=0):
    """AP with base's partition dim + custom free dims (stride, n)."""
    return bass.AP(base.tensor, base.offset + extra_off,
                   [list(base.ap[0])] + [list(d) for d in dims])


# --------------------------------------------------------------------------
# Host preprocessing
# --------------------------------------------------------------------------

def preprocess(inputs, cfg):
    c = cfg
    src = np.asarray(inputs["edge_index"][0], dtype=np.int64)
    dst = np.asarray(inputs["edge_index"][1], dtype=np.int64)
    offs = np.asarray(inputs["tweet_offsets"], dtype=np.int64)
    re_index = np.asarray(inputs["re_index"], dtype=np.int64)

    loop = np.arange(c.N, dtype=np.int64)
    srcA = np.concatenate([src, loop])
    dstA = np.concatenate([dst, loop])
    deg = np.bincount(dstA, minlength=c.N).astype(np.float64)
    dinv = (deg ** -0.5).astype(np.float32)
    normA = dinv[srcA] * dinv[dstA]

    core = dstA // c.NS
    dl = dstA - core * c.NS
    tl = dl // 128
    col = (dl - tl * 128).astype(np.float32)
    g = core * c.T + tl                      # global tile id, 0..C*T

    cnt = np.bincount(g, minlength=c.C * c.T)
    K = max(1, int(-(-cnt.max() // 128)))

    order = np.argsort(g, kind="stable")
    starts = np.zeros(c.C * c.T + 1, np.int64)
    np.cumsum(cnt, out=starts[1:])
    ranks = np.arange(len(g), dtype=np.int64) - starts[g[order]]
    p_ = ranks % 128
    k_ = ranks // 128
    go = g[order]
    co = go // c.T
    to = go - co * c.T

    # [C, T, 128, K] slot arrays
    sz = c.T * 128 * K
    dlcol = np.full((c.C, sz), -1.0, np.float32)
    snorm = np.zeros((c.C, sz), np.float32)
    srow = np.zeros((c.C, sz), np.int32)
    flat = (to * 128 + p_) * K + k_
    rowidx = ((srcA // c.NS) * c.NSP + (srcA % c.NS)).astype(np.int32)
    dlcol[co, flat] = col[order]
    snorm[co, flat] = normA[order]
    srow[co, flat] = rowidx[order]

    dl8 = np.ascontiguousarray(
        dlcol.reshape(c.C, c.T, 128, K).astype(np.int8))
    normh = np.ascontiguousarray(
        snorm.reshape(c.C, c.T, 128, K).astype(ml_dtypes.bfloat16))
    ridx = np.ascontiguousarray(srow.reshape(c.C, c.T, 128, K))

    # ---- user slots ----
    st = offs[re_index]
    ln = (offs[re_index + 1] - st).astype(np.int64)
    tot = int(ln.sum())
    uu = np.repeat(np.arange(c.U, dtype=np.int64), ln)
    csl = np.cumsum(ln) - ln
    pos = np.arange(tot, dtype=np.int64) - np.repeat(csl, ln)
    nodes = np.repeat(st, ln) + pos
    ucore = uu // c.UPC
    ulocal = uu - ucore * c.UPC
    uw = ulocal // 128
    ucol = (ulocal - uw * 128).astype(np.float32)
    gu = ucore * c.UW + uw
    ucnt = np.bincount(gu, minlength=c.C * c.UW)
    KU = max(1, int(-(-ucnt.max() // 128)))

    uorder = np.argsort(gu, kind="stable")
    ustarts = np.zeros(c.C * c.UW + 1, np.int64)
    np.cumsum(ucnt, out=ustarts[1:])
    uranks = np.arange(tot, dtype=np.int64) - ustarts[gu[uorder]]
    up_ = uranks % 128
    uk_ = uranks // 128
    guo = gu[uorder]
    uco = guo // c.UW
    uwo = guo - uco * c.UW

    usz = c.UW * 128 * KU
    umeta = np.full((c.C, usz), -1.0, np.float32)
    urow = np.zeros((c.C, usz), np.int32)
    uflat = (uwo * 128 + up_) * KU + uk_
    urowidx = ((nodes // c.NS) * c.NSP + (nodes % c.NS)).astype(np.int32)
    umeta[uco, uflat] = ucol[uorder]
    urow[uco, uflat] = urowidx[uorder]
    umeta = np.ascontiguousarray(umeta.reshape(c.C, c.UW, 128, KU))
    uridx = np.ascontiguousarray(urow.reshape(c.C, c.UW, 128, KU))

    return dict(K=K, KU=KU, dl8=dl8, normh=normh, ridx=ridx,
                umeta=umeta, uridx=uridx)


# --------------------------------------------------------------------------
# Bass program
# --------------------------------------------------------------------------

def build_program(cfg, K, KU, stage="full"):
    c = cfg
    nc = bacc.Bacc(num_devices=c.C)

    numT = nc.declare_dram_parameter("numT", [c.NUMP, c.T, 128], BF16, isOutput=False)
    catT = nc.declare_dram_parameter("catT", [c.CATP, c.T, 128], BF16, isOutput=False)
    wnum = nc.declare_dram_parameter("wnum", [c.NUMP, 32], BF16, isOutput=False)
    wcat = nc.declare_dram_parameter("wcat", [c.CATP, 32], BF16, isOutput=False)
    wtog = nc.declare_dram_parameter("wtog", [64, 64], F32, isOutput=False)
    brep = nc.declare_dram_parameter("brep", [128, 128], F32, isOutput=False)
    # brep cols: 0:32 b_num, 32:64 b_cat, 64:128 b_tog (replicated rows)
    attl = nc.declare_dram_parameter("attl", [64, 1], F32, isOutput=False)
    attr = nc.declare_dram_parameter("attr", [64, 1], F32, isOutput=False)
    dl_p = nc.declare_dram_parameter("dl8", [c.T, 128, K], I8, isOutput=False)
    norm_p = nc.declare_dram_parameter("normh", [c.T, 128, K], BF16, isOutput=False)
    ridx_p = nc.declare_dram_parameter("ridx", [c.T, 128, K], I32, isOutput=False)
    umeta_p = nc.declare_dram_parameter("umeta", [c.UW, 128, KU], F32, isOutput=False)
    uridx_p = nc.declare_dram_parameter("uridx", [c.UW, 128, KU], I32, isOutput=False)
    wf1 = nc.declare_dram_parameter("wf1", [64, 32], F32, isOutput=False)
    bf1 = nc.declare_dram_parameter("bf1", [32, 1], F32, isOutput=False)
    wlab = nc.declare_dram_parameter("wlab", [32, 2], F32, isOutput=False)
    blab = nc.declare_dram_parameter("blab", [2, 1], F32, isOutput=False)
    out_p = nc.declare_dram_parameter("out", [2, c.UW, 128], F32, isOutput=True)
    if stage == "enc":
        dbg = nc.declare_dram_parameter("dbg", [c.T * 128, 65], BF16, isOutput=True)
    elif stage == "ag0":
        dbg = nc.declare_dram_parameter("dbg", [c.TBLR, 65], BF16, isOutput=True)
    elif stage == "lay1":
        dbg = nc.declare_dram_parameter("dbg", [c.T * 128, 65], BF16, isOutput=True)

    iota_np = np.tile(np.arange(128, dtype=np.float32)[None, :], (128, 1))
    ident_np = np.eye(128, dtype=np.float32)
    ones1_np = np.ones((1, 128), np.float32)
    iota_d = nc.inline_tensor(iota_np, name="iota")
    ident_d = nc.inline_tensor(ident_np, name="ident")
    ones1_d = nc.inline_tensor(ones1_np, name="ones1")

    with tile.TileContext(nc) as tc:
        with tc.tile_pool(name="dram", bufs=1, space="DRAM") as dp:
            x0_dram = dp.tile([c.T, 128, 64], F32)
            tab0_own = dp.tile([c.T, 128, 65], BF16)
            tab1_own = dp.tile([c.T, 128, 65], BF16)
            tab2_own = dp.tile([c.T, 128, 64], BF16)
            arT0 = dp.tile([c.T, 1, 128], F32)
            arT1 = dp.tile([c.T, 1, 128], F32)
            tab0_full = dp.tile([c.TBLR, 65], BF16, addr_space="Shared")
            tab1_full = dp.tile([c.TBLR, 65], BF16, addr_space="Shared")
            tab2_full = dp.tile([c.TBLR, 64], BF16, addr_space="Shared")

            with tc.tile_pool(name="consts", bufs=1) as cp:
                iota_s = cp.tile([128, 128], F32)
                nc.sync.dma_start(out=iota_s[:], in_=iota_d[:, :])
                ident_s = cp.tile([128, 128], F32)
                nc.sync.dma_start(out=ident_s[:], in_=ident_d[:, :])
                ones1_s = cp.tile([1, 128], F32)
                nc.sync.dma_start(out=ones1_s[:], in_=ones1_d[:, :])
                wnum_s = cp.tile([c.NUMP, 32], BF16)
                nc.sync.dma_start(out=wnum_s[:], in_=wnum[:, :])
                wcat_s = cp.tile([c.CATP, 32], BF16)
                nc.sync.dma_start(out=wcat_s[:], in_=wcat[:, :])
                wtog_s = cp.tile([64, 64], F32)
                nc.sync.dma_start(out=wtog_s[:], in_=wtog[:, :])
                brep_s = cp.tile([128, 128], F32)
                nc.sync.dma_start(out=brep_s[:], in_=brep[:, :])
                attl_s = cp.tile([64, 1], F32)
                nc.sync.dma_start(out=attl_s[:], in_=attl[:, :])
                attr_s = cp.tile([64, 1], F32)
                nc.sync.dma_start(out=attr_s[:], in_=attr[:, :])
                wf1_s = cp.tile([64, 32], F32)
                nc.sync.dma_start(out=wf1_s[:], in_=wf1[:, :])
                bf1_s = cp.tile([32, 1], F32)
                nc.sync.dma_start(out=bf1_s[:], in_=bf1[:, :])
                wlab_s = cp.tile([32, 2], F32)
                nc.sync.dma_start(out=wlab_s[:], in_=wlab[:, :])
                blab_s = cp.tile([2, 1], F32)
                nc.sync.dma_start(out=blab_s[:], in_=blab[:, :])
                eps8 = cp.tile([128, 1], F32)
                nc.vector.memset(eps8[:], 1e-8)

                # ---------------- encoder ----------------
                with tc.tile_pool(name="enc", bufs=2) as ep, \
                     tc.tile_pool(name="encps", bufs=1, space="PSUM") as pp:
                    with tc.For_i(0, c.T) as i:
                        nt = ep.tile([c.NUMP, 128], BF16, tag="nt")
                        nc.sync.dma_start(
                            out=nt[:], in_=numT[:, ds(i, 1), :].opt())
                        ct = ep.tile([c.CATP, 128], BF16, tag="ct")
                        nc.sync.dma_start(
                            out=ct[:], in_=catT[:, ds(i, 1), :].opt())
                        xc = ep.tile([128, 64], F32, tag="xc")
                        p1 = pp.tile([128, 32], F32, tag="p1")
                        nc.tensor.matmul(out=p1[:], lhsT=nt[:], rhs=wnum_s[:],
                                         start=True, stop=True)
                        t1 = ep.tile([128, 32], F32, tag="t1")
                        nc.vector.tensor_tensor(out=t1[:], in0=p1[:],
                                                in1=brep_s[:, 0:32], op=OP.add)
                        nc.vector.scalar_tensor_tensor(
                            out=xc[:, 0:32], in0=t1[:], scalar=SLOPE,
                            in1=t1[:], op0=OP.mult, op1=OP.max)
                        p2 = pp.tile([128, 32], F32, tag="p2")
                        nc.tensor.matmul(out=p2[:], lhsT=ct[:], rhs=wcat_s[:],
                                         start=True, stop=True)
                        t2 = ep.tile([128, 32], F32, tag="t2")
                        nc.vector.tensor_tensor(out=t2[:], in0=p2[:],
                                                in1=brep_s[:, 32:64], op=OP.add)
                        nc.vector.scalar_tensor_tensor(
                            out=xc[:, 32:64], in0=t2[:], scalar=SLOPE,
                            in1=t2[:], op0=OP.mult, op1=OP.max)
                        # x = lrelu(xc @ wtog + b_tog)
                        pt = pp.tile([64, 128], F32, tag="pt")
                        nc.tensor.transpose(out=pt[:], in_=xc[:],
                                            identity=ident_s[:])
                        xcT = ep.tile([64, 128], F32, tag="xcT")
                        nc.scalar.copy(out=xcT[:], in_=pt[:])
                        p3 = pp.tile([128, 64], F32, tag="p3")
                        nc.tensor.matmul(out=p3[:], lhsT=xcT[:], rhs=wtog_s[:],
                                         start=True, stop=True)
                        t3 = ep.tile([128, 64], F32, tag="t3")
                        nc.vector.tensor_tensor(out=t3[:], in0=p3[:],
                                                in1=brep_s[:, 64:128], op=OP.add)
                        x0s = ep.tile([128, 64], F32, tag="x0s")
                        nc.vector.scalar_tensor_tensor(
                            out=x0s[:], in0=t3[:], scalar=SLOPE,
                            in1=t3[:], op0=OP.mult, op1=OP.max)
                        nc.sync.dma_start(
                            out=x0_dram[ds(i, 1)].opt(), in_=x0s[:])
                        # al0 / arT0 / bf16 table row
                        ph = pp.tile([64, 128], F32, tag="ph")
                        nc.tensor.transpose(out=ph[:], in_=x0s[:],
                                            identity=ident_s[:])
                        hT = ep.tile([64, 128], F32, tag="hT")
                        nc.scalar.copy(out=hT[:], in_=ph[:])
                        pal = pp.tile([128, 1], F32, tag="pal")
                        nc.tensor.matmul(out=pal[:], lhsT=hT[:], rhs=attl_s[:],
                                         start=True, stop=True)
                        tb = ep.tile([128, 65], BF16, tag="tb")
                        nc.scalar.copy(out=tb[:, 0:1], in_=pal[:])
                        nc.vector.tensor_copy(out=tb[:, 1:65], in_=x0s[:])
                        nc.sync.dma_start(
                            out=tab0_own[ds(i, 1)].opt(), in_=tb[:])
                        par = pp.tile([1, 128], F32, tag="par")
                        nc.tensor.matmul(out=par[:], lhsT=attr_s[:], rhs=hT[:],
                                         start=True, stop=True)
                        ars = ep.tile([1, 128], F32, tag="arsenc")
                        nc.scalar.copy(out=ars[:], in_=par[:])
                        nc.sync.dma_start(
                            out=arT0[ds(i, 1)].opt(), in_=ars[:])

                if stage == "enc":
                    nc.sync.dma_start(
                        out=dbg[:, :],
                        in_=tab0_own[:].rearrange("t p f -> (t p) f"))
                if stage not in ("enc",):
                    nc.gpsimd.collective_compute(
                        "AllGather", OP.bypass,
                        replica_groups=[list(range(c.C))],
                        ins=[tab0_own[:].rearrange("t p f -> (t p) f").opt()],
                        outs=[tab0_full[:].opt()])
                if stage == "ag0":
                    nc.sync.dma_start(out=dbg[:, :], in_=tab0_full[:, :])

                # ---------------- FAConv layers ----------------
                def layer(tab_full, arT_dram, layer2):
                    with tc.tile_pool(name="lay", bufs=2) as lp, \
                         tc.tile_pool(name="layps", bufs=1, space="PSUM") as qp:
                        with tc.For_i(0, c.T) as i:
                            dl8s = lp.tile([128, K], I8, tag="dl8s")
                            nc.sync.dma_start(
                                out=dl8s[:], in_=dl_p[ds(i, 1)].opt())
                            ms = lp.tile([128, K], F32, tag="ms")
                            nc.vector.tensor_copy(out=ms[:], in_=dl8s[:])
                            nh = lp.tile([128, K], BF16, tag="nh")
                            nc.sync.dma_start(
                                out=nh[:], in_=norm_p[ds(i, 1)].opt())
                            rs = lp.tile([128, K], I32, tag="rs")
                            nc.scalar.dma_start(
                                out=rs[:], in_=ridx_p[ds(i, 1)].opt())
                            ars = lp.tile([1, 128], F32, tag="ars")
                            nc.sync.dma_start(
                                out=ars[:], in_=arT_dram[ds(i, 1)].opt())
                            hg = lp.tile([128, K * 65], BF16, tag="hg")
                            for k in range(K):
                                nc.gpsimd.indirect_dma_start(
                                    out=hg[:, k * 65:(k + 1) * 65],
                                    out_offset=None,
                                    in_=tab_full[:, :],
                                    in_offset=bass.IndirectOffsetOnAxis(
                                        ap=rs[:, k:k + 1], axis=0))
                            # one-hot m01 over dst column
                            m01 = lp.tile([128, K, 128], F32, tag="m01")
                            nc.vector.tensor_tensor(
                                out=m01[:],
                                in0=_ap(ms[:], [[1, K], [0, 128]]),
                                in1=_ap(iota_s[:], [[0, K], [1, 128]]),
                                op=OP.is_equal)
                            # ar broadcast to [128,128] rows via ones matmul
                            prep = qp.tile([128, 128], F32, tag="prep")
                            nc.tensor.matmul(out=prep[:], lhsT=ones1_s[:],
                                             rhs=ars[:], start=True, stop=True)
                            arm = lp.tile([128, K, 128], F32, tag="arm")
                            nc.vector.tensor_tensor(
                                out=arm[:], in0=m01[:],
                                in1=_ap(prep[:], [[0, K], [1, 128]]),
                                op=OP.mult)
                            arslot = lp.tile([128, K], F32, tag="arslot")
                            nc.vector.tensor_reduce(
                                out=arslot[:], in_=arm[:], axis=AX.X,
                                op=OP.add)
                            # alpha = tanh(al_src + ar_dst) * norm
                            tsum = lp.tile([128, K], F32, tag="tsum")
                            nc.vector.tensor_tensor(
                                out=tsum[:],
                                in0=_ap(hg[:], [[65, K]]),
                                in1=arslot[:], op=OP.add)
                            th = lp.tile([128, K], F32, tag="th")
                            nc.scalar.activation(out=th[:], in_=tsum[:],
                                                 func=AF.Tanh)
                            alpha = lp.tile([128, K], F32, tag="alpha")
                            nc.vector.tensor_tensor(
                                out=alpha[:], in0=th[:], in1=nh[:],
                                op=OP.mult)
                            la = lp.tile([128, K, 128], BF16, tag="la")
                            nc.vector.tensor_tensor(
                                out=la[:], in0=m01[:],
                                in1=_ap(alpha[:], [[1, K], [0, 128]]),
                                op=OP.mult)
                            agg = qp.tile([128, 64], F32, tag="agg")
                            for k in range(K):
                                nc.tensor.matmul(
                                    out=agg[:], lhsT=la[:, k, :],
                                    rhs=hg[:, k * 65 + 1:(k + 1) * 65],
                                    start=(k == 0), stop=(k == K - 1))
                            x0b = lp.tile([128, 64], F32, tag="x0b")
                            nc.scalar.dma_start(
                                out=x0b[:], in_=x0_dram[ds(i, 1)].opt())
                            xo = lp.tile([128, 64], F32, tag="xo")
                            nc.vector.scalar_tensor_tensor(
                                out=xo[:], in0=x0b[:], scalar=EPS,
                                in1=agg[:], op0=OP.mult, op1=OP.add)
                            if not layer2:
                                ph2 = qp.tile([64, 128], F32, tag="ph2")
                                nc.tensor.transpose(out=ph2[:], in_=xo[:],
                                                    identity=ident_s[:])
                                hT2 = lp.tile([64, 128], F32, tag="hT2")
                                nc.scalar.copy(out=hT2[:], in_=ph2[:])
                                pal2 = qp.tile([128, 1], F32, tag="pal2")
                                nc.tensor.matmul(out=pal2[:], lhsT=hT2[:],
                                                 rhs=attl_s[:],
                                                 start=True, stop=True)
                                tb1 = lp.tile([128, 65], BF16, tag="tb1")
                                nc.scalar.copy(out=tb1[:, 0:1], in_=pal2[:])
                                nc.vector.tensor_copy(out=tb1[:, 1:65],
                                                      in_=xo[:])
                                nc.sync.dma_start(
                                    out=tab1_own[ds(i, 1)].opt(), in_=tb1[:])
                                par2 = qp.tile([1, 128], F32, tag="par2")
                                nc.tensor.matmul(out=par2[:], lhsT=attr_s[:],
                                                 rhs=hT2[:],
                                                 start=True, stop=True)
                                ar2 = lp.tile([1, 128], F32, tag="ar2")
                                nc.scalar.copy(out=ar2[:], in_=par2[:])
                                nc.sync.dma_start(
                                    out=arT1[ds(i, 1)].opt(), in_=ar2[:])
                            else:
                                sq = lp.tile([128, 64], F32, tag="sq")
                                nc.vector.tensor_tensor(
                                    out=sq[:], in0=xo[:], in1=xo[:],
                                    op=OP.mult)
                                tb2 = lp.tile([128, 64], BF16, tag="tb2")
                                nc.scalar.activation(out=tb2[:], in_=sq[:],
                                                     func=AF.Sqrt,
                                                     bias=eps8[:, 0:1])
                                nc.sync.dma_start(
                                    out=tab2_own[ds(i, 1)].opt(), in_=tb2[:])

                if stage not in ("enc", "ag0"):
                    layer(tab0_full, arT0, layer2=False)
                if stage == "lay1":
                    nc.sync.dma_start(
                        out=dbg[:, :],
                        in_=tab1_own[:].rearrange("t p f -> (t p) f"))
                if stage == "full":
                    nc.gpsimd.collective_compute(
                        "AllGather", OP.bypass,
                        replica_groups=[list(range(c.C))],
                        ins=[tab1_own[:].rearrange("t p f -> (t p) f").opt()],
                        outs=[tab1_full[:].opt()])
                if stage == "full":
                    layer(tab1_full, arT1, layer2=True)
                    nc.gpsimd.collective_compute(
                        "AllGather", OP.bypass,
                        replica_groups=[list(range(c.C))],
                        ins=[tab2_own[:].rearrange("t p f -> (t p) f").opt()],
                        outs=[tab2_full[:].opt()])

                # ---------------- users + MLP ----------------
                with tc.tile_pool(name="usr", bufs=2) as up2, \
                     tc.tile_pool(name="usrps", bufs=1, space="PSUM") as vp:
                    with tc.For_i(0, c.UW) as i:
                        um = up2.tile([128, KU], F32, tag="um")
                        nc.sync.dma_start(
                            out=um[:], in_=umeta_p[ds(i, 1)].opt())
                        ur = up2.tile([128, KU], I32, tag="ur")
                        nc.scalar.dma_start(
                            out=ur[:], in_=uridx_p[ds(i, 1)].opt())
                        ug = up2.tile([128, KU * 64], BF16, tag="ug")
                        for k in range(KU):
                            nc.gpsimd.indirect_dma_start(
                                out=ug[:, k * 64:(k + 1) * 64],
                                out_offset=None,
                                in_=tab2_full[:, :],
                                in_offset=bass.IndirectOffsetOnAxis(
                                    ap=ur[:, k:k + 1], axis=0))
                        m01u = up2.tile([128, KU, 128], BF16, tag="m01u")
                        nc.vector.tensor_tensor(
                            out=m01u[:],
                            in0=_ap(um[:], [[1, KU], [0, 128]]),
                            in1=_ap(iota_s[:], [[0, KU], [1, 128]]),
                            op=OP.is_equal)
                        psy = vp.tile([128, 64], F32, tag="psy")
                        for k in range(KU):
                            nc.tensor.matmul(
                                out=psy[:], lhsT=m01u[:, k, :],
                                rhs=ug[:, k * 64:(k + 1) * 64],
                                start=(k == 0), stop=(k == KU - 1))
                        ys = up2.tile([128, 64], F32, tag="ys")
                        nc.scalar.copy(out=ys[:], in_=psy[:])
                        pyt = vp.tile([64, 128], F32, tag="pyt")
                        nc.tensor.transpose(out=pyt[:], in_=ys[:],
                                            identity=ident_s[:])
                        yts = up2.tile([64, 128], F32, tag="yts")
                        nc.scalar.copy(out=yts[:], in_=pyt[:])
                        h1p = vp.tile([32, 128], F32, tag="h1p")
                        nc.tensor.matmul(out=h1p[:], lhsT=wf1_s[:], rhs=yts[:],
                                         start=True, stop=True)
                        h1b = up2.tile([32, 128], F32, tag="h1b")
                        nc.scalar.activation(out=h1b[:], in_=h1p[:],
                                             func=AF.Identity,
                                             bias=bf1_s[:, 0:1])
                        h1s = up2.tile([32, 128], F32, tag="h1s")
                        nc.vector.scalar_tensor_tensor(
                            out=h1s[:], in0=h1b[:], scalar=SLOPE,
                            in1=h1b[:], op0=OP.mult, op1=OP.max)
                        o2p = vp.tile([2, 128], F32, tag="o2p")
                        nc.tensor.matmul(out=o2p[:], lhsT=wlab_s[:],
                                         rhs=h1s[:], start=True, stop=True)
                        o2s = up2.tile([2, 128], F32, tag="o2s")
                        nc.scalar.activation(out=o2s[:], in_=o2p[:],
                                             func=AF.Identity,
                                             bias=blab_s[:, 0:1])
                        nc.sync.dma_start(
                            out=out_p[:, ds(i, 1), :].opt(), in_=o2s[:])
    nc.finalize()
    return nc


# --------------------------------------------------------------------------
# Entry point
# --------------------------------------------------------------------------

_CACHE = {}


def _prog(key, builder, *args):
    if key not in _CACHE:
        _CACHE[key] = builder(*args)
    return _CACHE[key]


def make_in_maps(inputs, cfg, meta):
    c = cfg
    num = np.asarray(inputs["num_prop"], np.float32)
    cat = np.asarray(inputs["cat_prop"], np.float32)
    numP = np.zeros((c.NSP * c.C, c.NUMP), np.float32)
    catP = np.zeros((c.NSP * c.C, c.CATP), np.float32)
    for cc in range(c.C):
        numP[cc * c.NSP:cc * c.NSP + c.NS] = num[cc * c.NS:(cc + 1) * c.NS]
        catP[cc * c.NSP:cc * c.NSP + c.NS] = cat[cc * c.NS:(cc + 1) * c.NS]
    numT = numP.reshape(c.C, c.T, 128, c.NUMP).transpose(0, 3, 1, 2)\
        .astype(ml_dtypes.bfloat16)
    catT = catP.reshape(c.C, c.T, 128, c.CATP).transpose(0, 3, 1, 2)\
        .astype(ml_dtypes.bfloat16)

    brep = np.zeros((128, 128), np.float32)
    brep[:, 0:32] = np.asarray(inputs["b_num"], np.float32)[None, :]
    brep[:, 32:64] = np.asarray(inputs["b_cat"], np.float32)[None, :]
    brep[:, 64:128] = np.asarray(inputs["b_tog"], np.float32)[None, :]

    com = dict(
        wnum=np.ascontiguousarray(
            np.asarray(inputs["W_num"], np.float32).astype(ml_dtypes.bfloat16)),
        wcat=np.ascontiguousarray(
            np.asarray(inputs["W_cat"], np.float32).astype(ml_dtypes.bfloat16)),
        wtog=np.ascontiguousarray(np.asarray(inputs["W_tog"], np.float32)),
        brep=brep,
        attl=np.ascontiguousarray(
            np.asarray(inputs["att_l"], np.float32).reshape(64, 1)),
        attr=np.ascontiguousarray(
            np.asarray(inputs["att_r"], np.float32).reshape(64, 1)),
        wf1=np.ascontiguousarray(np.asarray(inputs["W_f1"], np.float32)),
        bf1=np.ascontiguousarray(
            np.asarray(inputs["b_f1"], np.float32).reshape(32, 1)),
        wlab=np.ascontiguousarray(np.asarray(inputs["W_lab"], np.float32)),
        blab=np.ascontiguousarray(
            np.asarray(inputs["b_lab"], np.float32).reshape(2, 1)),
    )
    maps = []
    for cc in range(c.C):
        m = dict(com)
        m["numT"] = np.ascontiguousarray(numT[cc])
        m["catT"] = np.ascontiguousarray(catT[cc])
        m["dl8"] = meta["dl8"][cc]
        m["normh"] = meta["normh"][cc]
        m["ridx"] = meta["ridx"][cc]
        m["umeta"] = meta["umeta"][cc]
        m["uridx"] = meta["uridx"][cc]
        maps.append(m)
    return maps


def run_all(inputs, cfg, runner):
    c = cfg
    meta = preprocess(inputs, cfg)
    nc = _prog(("main", c.N, c.U, meta["K"], meta["KU"]), build_program,
               cfg, meta["K"], meta["KU"])
    maps = make_in_maps(inputs, cfg, meta)
    res = runner(nc, maps)
    out = np.zeros((c.U, 2), np.float32)
    for cc in range(c.C):
        out[cc * c.UPC:(cc + 1) * c.UPC] = \
            res[cc]["out"].reshape(2, c.UPCP)[:, :c.UPC].T
    return out


def kernel(**inputs):
    import time
    from concourse.bass_utils import run_bass_kernel_spmd
    cfg = Cfg()

    def runner(nc, in_maps):
        last = None
        for attempt in range(3):
            try:
                return run_bass_kernel_spmd(
                    nc, in_maps, core_ids=list(range(cfg.C))).results
            except Exception as e:  # transient NRT faults: retry
                last = e
                time.sleep(5.0)
        raise last

    return run_all(inputs, cfg, runner)
